# revision 3
# baseline (speedup 1.0000x reference)
"""GQA causal attention (QKV proj + NeoX RoPE + softmax attention + o_proj)
for Trainium2, tensor-parallel over heads across 8 NeuronCores.  bf16 v2.

Problem shapes (hardcoded): B=1, S=2048, HID=2048, NH=32, NKV=8, HD=64.
Per core c: 4 query heads (4c..4c+3) + 1 kv head (c).

All matmul operands bf16 (1 cyc/row on PE, same as fp32r, but half the DMA
bytes and 4x DVE throughput on SBUF elementwise ops).  PSUM accum fp32.

Single fused schedule (Tile overlaps everything by data deps):
  loads -> qkv(c0) -> rope(c0) -> attn(j0) -> qkv(c1)+rope(c1)
        -> o_proj(j0) -> attn(j1) -> o_proj(j1)
PSUM budget: ph0 pool (6 banks, scoped to c0) released, then
  gemm(2) + st(2x2) + pv(2) = 8 banks.
"""

import numpy as np

import concourse.bass as bass
import concourse.mybir as mybir
import concourse.tile as tile
from concourse import bacc
from concourse.masks import make_identity

B, S, HID = 1, 2048, 2048
NH, NKV, HD = 32, 8, 64
NCORES = 8
HPC = NH // NCORES          # 4 query heads per core
ROPE_BASE = 10000.0
SCALE = 1.0 / np.sqrt(HD)   # 0.125
NEG = -1e9

F32 = mybir.dt.float32
BF16 = mybir.dt.bfloat16

KT = S // 128               # 16 k-tiles
MC = 1024                   # m-chunk / q-chunk
NMC = S // MC               # 2


def _chunks(total, step=512):
    out = []
    o = 0
    while o < total:
        out.append((o, min(step, total - o)))
        o += step
    return out


def build_kernel(passes=1, upto="full"):
    nc = bacc.Bacc("TRN2", target_bir_lowering=False, debug=False,
                   num_devices=NCORES)

    xT = nc.dram_tensor("xT", [HID, S], BF16, kind="ExternalInput").ap()
    w_stat = nc.dram_tensor("w_stat", [HID, 384], BF16,
                            kind="ExternalInput").ap()
    w_o = nc.dram_tensor("w_o", [256, HID], BF16, kind="ExternalInput").ap()
    Cr = nc.dram_tensor("C", [128, S], BF16, kind="ExternalInput").ap()
    Sr = nc.dram_tensor("Sn", [128, S], BF16, kind="ExternalInput").ap()
    maskneg = nc.dram_tensor("maskneg", [128, 128], BF16,
                             kind="ExternalInput").ap()
    yT = nc.dram_tensor("yT", [HID, S], BF16, kind="ExternalOutput").ap()

    with tile.TileContext(nc) as tc:
      for _pass in range(passes):
        with (
            tc.tile_pool(name="pers", bufs=1, side=None) as pers,
            tc.tile_pool(name="swp", bufs=2) as swp,
            tc.tile_pool(name="rtmp", bufs=2) as rtmp,
            tc.tile_pool(name="ptp", bufs=8) as ptp,
            tc.tile_pool(name="sumsp", bufs=2) as sumsp,
            tc.tile_pool(name="recp", bufs=2) as recp,
            tc.tile_pool(name="otp", bufs=2) as otp,
            tc.tile_pool(name="ysbp", bufs=4) as ysbp,
        ):
            # ---------------- persistent tiles ----------------
            wt = [pers.tile([128, 384], BF16, tag=f"wt{k}", name=f"wt{k}")
                  for k in range(KT)]
            xsb = [[pers.tile([128, MC], BF16, tag=f"x{k}_{c}",
                              name=f"x{k}_{c}") for c in range(NMC)]
                   for k in range(KT)]
            # qr[p][c]: roped q heads pair p, chunk c.  kr[c]: rows 64:128
            # roped kT, rows 0:64 duplicate.
            qr = [[pers.tile([128, MC], BF16, tag=f"qr{p}_{c}",
                             name=f"qr{p}_{c}") for c in range(NMC)]
                  for p in range(2)]
            kr = [pers.tile([128, MC], BF16, tag=f"kr{c}", name=f"kr{c}")
                  for c in range(NMC)]
            vaug = [pers.tile([128, 128], BF16, tag=f"va{i}", name=f"va{i}")
                    for i in range(KT)]
            outstat = [[pers.tile([128, MC], BF16, tag=f"os{p}_{j}",
                                  name=f"os{p}_{j}") for j in range(NMC)]
                       for p in range(2)]
            wo_sb = [pers.tile([128, HID], BF16, tag=f"wo{p}", name=f"wo{p}")
                     for p in range(2)]
            Ct = pers.tile([128, S], BF16, tag="Ct")
            St = pers.tile([128, S], BF16, tag="St")
            mneg = pers.tile([128, 128], BF16, tag="mneg")
            ident = pers.tile([128, 128], BF16, tag="ident")
            qkv = [[pers.tile([128, MC], BF16, tag=f"qkv{t}_{c}",
                              name=f"qkv{t}_{c}") for c in range(NMC)]
                   for t in range(3)]

            # ---------------- loads ----------------
            # interleave w and x chunk-0 loads so early k tiles arrive first
            for k in range(KT):
                nc.sync.dma_start(wt[k],
                                  w_stat[128 * k:128 * (k + 1), :])
                eng = nc.sync if k % 2 == 0 else nc.gpsimd
                eng.dma_start(xsb[k][0], xT[128 * k:128 * (k + 1), 0:MC])
            nc.gpsimd.dma_start(mneg, maskneg)
            make_identity(nc, ident)
            for i in range(KT):
                nc.gpsimd.memset(vaug[i][:, 64:128], 1.0)
            for k in range(KT):
                eng = nc.sync if k % 2 == 0 else nc.gpsimd
                eng.dma_start(xsb[k][1], xT[128 * k:128 * (k + 1), MC:S])
            for p in range(2):
                nc.sync.dma_start(wo_sb[p], w_o[128 * p:128 * (p + 1), :])
            nc.gpsimd.dma_start(Ct, Cr)
            nc.gpsimd.dma_start(St, Sr)

            def emit_qkv_n(mc, n, psum_pool, evict_eng):
                """One 128-col slice of qkvT for m-chunk mc."""
                for (c0, cl) in _chunks(MC):
                    ps = psum_pool.tile([128, 512], F32, tag="g",
                                        name=f"qps{mc}_{n}_{c0}")
                    for k in range(KT):
                        nc.tensor.matmul(
                            ps[:, 0:cl],
                            wt[k][:, 128 * n:128 * (n + 1)],
                            xsb[k][mc][:, c0:c0 + cl],
                            start=(k == 0), stop=(k == KT - 1))
                    evict_eng(qkv[n][mc][:, c0:c0 + cl], ps[:, 0:cl])

            def emit_qkv_kmajor(mc, psum_pool, evict_engs):
                """All 6 qkvT psum slices at once: k-major over the first 12
                k-tiles (so the PE can consume each x tile as its DMA lands),
                then per-slice completion so evictions overlap the tail."""
                KSPLIT = 12
                pss = {}
                for n in range(3):
                    for (c0, cl) in _chunks(MC):
                        pss[(n, c0)] = psum_pool.tile(
                            [128, 512], F32, tag="g", name=f"qps{mc}_{n}_{c0}")
                for k in range(KSPLIT):
                    for n in (2, 0, 1):
                        for (c0, cl) in _chunks(MC):
                            nc.tensor.matmul(
                                pss[(n, c0)][:, 0:cl],
                                wt[k][:, 128 * n:128 * (n + 1)],
                                xsb[k][mc][:, c0:c0 + cl],
                                start=(k == 0), stop=False)
                ei = 0
                for n in (2, 0, 1):
                    for (c0, cl) in _chunks(MC):
                        for k in range(KSPLIT, KT):
                            nc.tensor.matmul(
                                pss[(n, c0)][:, 0:cl],
                                wt[k][:, 128 * n:128 * (n + 1)],
                                xsb[k][mc][:, c0:c0 + cl],
                                start=False, stop=(k == KT - 1))
                        evict_engs[ei % len(evict_engs)](
                            qkv[n][mc][:, c0:c0 + cl], pss[(n, c0)][:, 0:cl])
                        ei += 1

            def emit_rope(mc, t):
                """RoPE for tile t (0,1 = q pairs; 2 = kT rows 64:128)."""
                src = qkv[t][mc]
                r0 = 0 if t < 2 else 64
                m0 = MC * mc
                sw = swp.tile([128, MC], BF16, tag="sw", name=f"sw{t}_{mc}")
                for g in range(r0 // 32, 4, 2):
                    nc.gpsimd.dma_start(sw[32 * g:32 * g + 32, :],
                                        src[32 * g + 32:32 * g + 64, :])
                    nc.gpsimd.dma_start(sw[32 * g + 32:32 * g + 64, :],
                                        src[32 * g:32 * g + 32, :])
                t1 = rtmp.tile([128, MC], BF16, tag="t1", name=f"t1_{t}_{mc}")
                t2 = rtmp.tile([128, MC], BF16, tag="t2", name=f"t2_{t}_{mc}")
                nc.vector.tensor_mul(t1[r0:128, :], src[r0:128, :],
                                     Ct[r0:128, m0:m0 + MC])
                nc.vector.tensor_mul(t2[r0:128, :], sw[r0:128, :],
                                     St[r0:128, m0:m0 + MC])
                dst = qr[t][mc] if t < 2 else kr[mc]
                nc.vector.tensor_add(dst[r0:128, :], t1[r0:128, :],
                                     t2[r0:128, :])
                if t == 2:
                    nc.gpsimd.dma_start(kr[mc][0:64, :], kr[mc][64:128, :])

            def emit_transposes(mc, psum_pool):
                for ic in range(8):
                    i = 8 * mc + ic
                    tp = psum_pool.tile([128, 64], BF16, tag="g",
                                        name=f"tr{i}")
                    nc.tensor.transpose(
                        tp, qkv[2][mc][0:64, 128 * ic:128 * (ic + 1)],
                        ident[0:64, 0:64])
                    nc.vector.tensor_copy(vaug[i][:, 0:64], tp)

            def emit_attn(j, stpool, pvpool, fillers=(), order=None,
                          microfill=None):
                """Global 2-deep software pipeline over all (head, i) tiles:
                pv matmuls lag their exp by two st tiles, and a head's
                normalize + boundary fillers fire after the next head's
                first tiles, so neither PE nor Act idles at boundaries."""
                order = list(order) if order is not None else list(range(HPC))
                ilast = 8 * (j + 1) - 1
                tiles = [(hx, h, i) for hx, h in enumerate(order)
                         for i in range(ilast + 1)]
                from collections import deque
                pend = deque()      # (pv, segs, done_fn or None)
                pvs = {}
                nseen = 0

                def flush_one():
                    pv, segs, done_fn = pend.popleft()
                    for (a0, al, pi, ppt, pqoff, stp) in segs:
                        nc.tensor.matmul(
                            pv[:, a0:a0 + al], vaug[pi],
                            ppt[:, a0 - pqoff:a0 - pqoff + al],
                            start=(pi == 0), stop=stp)
                    if done_fn is not None:
                        done_fn()

                def make_norm(hx, h, p, half, pv, last):
                    def norm():
                        rec = sumsp.tile([128, MC], F32, tag="rec",
                                         name=f"rcc{j}_{h}")
                        sums0 = sumsp.tile([64, MC], F32, tag="sums0",
                                           name=f"sm{j}_{h}")
                        rec0 = recp.tile([64, MC], F32, tag="rec0",
                                         name=f"rc{j}_{h}")
                        ot = (None if half == 0 else
                              otp.tile([64, MC], BF16, tag="ot",
                                       name=f"ot{j}_{h}"))
                        for (c0, cl) in _chunks(MC, 512 if last else MC):
                            nc.vector.tensor_copy(rec[64:128, c0:c0 + cl],
                                                  pv[64:128, c0:c0 + cl])
                            nc.gpsimd.dma_start(sums0[:, c0:c0 + cl],
                                                rec[64:128, c0:c0 + cl])
                            nc.vector.reciprocal_approx_fast(
                                rec0[:, c0:c0 + cl], sums0[:, c0:c0 + cl])
                            if half == 0:
                                nc.vector.tensor_mul(
                                    outstat[p][j][0:64, c0:c0 + cl],
                                    pv[0:64, c0:c0 + cl],
                                    rec0[:, c0:c0 + cl])
                            else:
                                nc.vector.tensor_mul(ot[:, c0:c0 + cl],
                                                     pv[0:64, c0:c0 + cl],
                                                     rec0[:, c0:c0 + cl])
                                nc.gpsimd.dma_start(
                                    outstat[p][j][64:128, c0:c0 + cl],
                                    ot[:, c0:c0 + cl])
                        if hx < len(fillers):
                            for fn in fillers[hx]:
                                fn()
                    return norm

                for (hx, h, i) in tiles:
                    p, half = h // 2, h % 2
                    qrow = 64 * half
                    kb = qrow
                    if i == 0:
                        pvs[h] = pvpool.tile([128, MC], F32, tag="pv",
                                             name=f"pv{j}_{h}")
                    pv = pvs[h]
                    kc, ic = i // 8, i % 8
                    qstart = max(MC * j, 128 * i)
                    qlen = MC * (j + 1) - qstart
                    qoff = qstart - MC * j
                    diag = 128 * i >= MC * j
                    st = stpool.tile([128, MC], F32, tag="st",
                                     name=f"st{j}_{h}_{i}")
                    kt_ap = kr[kc][kb:kb + 64, 128 * ic:128 * (ic + 1)]
                    if diag:
                        nc.tensor.matmul(st[:, 0:128], ident, mneg,
                                         start=True, stop=False)
                        nc.tensor.matmul(
                            st[:, 0:128], kt_ap,
                            qr[p][j][qrow:qrow + 64, qoff:qoff + 128],
                            start=False, stop=True)
                        rest = ([(128, min(qlen, 512) - 128)]
                                if min(qlen, 512) > 128 else [])
                        rest += [(c0, cl) for (c0, cl) in _chunks(qlen)
                                 if c0 >= 512]
                    else:
                        rest = _chunks(qlen)
                    for (c0, cl) in rest:
                        nc.tensor.matmul(
                            st[:, c0:c0 + cl], kt_ap,
                            qr[p][j][qrow:qrow + 64,
                                     qoff + c0:qoff + c0 + cl],
                            start=True, stop=True)
                    while len(pend) >= 2:
                        flush_one()
                    pt = ptp.tile([128, MC], BF16, tag="pt",
                                  name=f"pt{j}_{h}_{i}")
                    nc.scalar.activation(
                        pt[:, 0:qlen], st[:, 0:qlen],
                        mybir.ActivationFunctionType.Exp, scale=SCALE)
                    segs, a = [], qoff
                    while a < MC:
                        nxt = min(MC, (a // 512 + 1) * 512)
                        lasti = (8 * j + 3) if a < 512 else ilast
                        segs.append((a, nxt - a, i, pt, qoff, i == lasti))
                        a = nxt
                    done = None
                    if i == ilast:
                        done = make_norm(hx, h, p, half, pv,
                                         hx == len(order) - 1)
                    pend.append((pv, segs, done))
                    nseen += 1
                    if microfill is not None and nseen % 2 == 0:
                        u = next(microfill, None)
                        if u is not None:
                            u()
                while pend:
                    flush_one()

            def emit_oproj_unit(j, nt, c0, cl, psum_pool, evict_eng, ei):
                ps = psum_pool.tile([128, 512], F32, tag="g",
                                    name=f"ops{j}_{nt}_{c0}")
                for p in range(2):
                    nc.tensor.matmul(
                        ps[:, 0:cl],
                        wo_sb[p][:, 128 * nt:128 * (nt + 1)],
                        outstat[p][j][:, c0:c0 + cl],
                        start=(p == 0), stop=(p == 1))
                ysb = ysbp.tile([128, 512], BF16, tag="y",
                                name=f"ysb{j}_{nt}_{c0}")
                evict_eng(ysb, ps[:, 0:cl])
                dmae = nc.sync if ei % 2 == 0 else nc.gpsimd
                dmae.dma_start(
                    yT[128 * nt:128 * (nt + 1),
                       MC * j + c0:MC * j + c0 + cl], ysb)

            def emit_oproj(j, psum_pool, evict_engs, nts=None):
                ei = 0
                for nt in (range(KT) if nts is None else nts):
                    for (c0, cl) in _chunks(MC):
                        emit_oproj_unit(j, nt, c0, cl, psum_pool,
                                        evict_engs[ei % len(evict_engs)], ei)
                        ei += 1

            # ---------------- chunk 0 (6-bank scoped psum) ----------------
            with tc.tile_pool(name="ph0ps", bufs=6, space="PSUM") as ph0ps:
                emit_qkv_kmajor(0, ph0ps,
                                [nc.scalar.copy, nc.vector.tensor_copy])
                emit_rope(0, 2)
                emit_rope(0, 0)
                emit_transposes(0, ph0ps)
                emit_rope(0, 1)

            # ---------------- rest: 2+4+2 banks ----------------
            with (
                tc.tile_pool(name="gemmps", bufs=2, space="PSUM") as gemmps,
                tc.tile_pool(name="stps", bufs=2, space="PSUM") as stps,
                tc.tile_pool(name="pvps", bufs=1, space="PSUM") as pvps,
            ):
                # attn j=0 with chunk-1 qkv work interleaved at head ends
                # (PE filler while Act drains the exp stream).
                c1ev = [nc.vector.tensor_copy, nc.scalar.copy]
                c1_fill = [
                    [lambda: emit_qkv_n(1, 2, gemmps, c1ev[0]),
                     lambda: emit_rope(1, 2)],
                    [lambda: emit_qkv_n(1, 0, gemmps, c1ev[1]),
                     lambda: emit_rope(1, 0)],
                    [lambda: emit_transposes(1, gemmps)],
                    [lambda: emit_qkv_n(1, 1, gemmps, c1ev[0]),
                     lambda: emit_rope(1, 1)],
                ]
                emit_attn(0, stps, pvps, c1_fill, order=(1, 0, 3, 2))

                # attn j=1 with o_proj(j=0) micro-interleaved into the i-loop
                # (one 2-matmul psum unit per i-tile; evictions on DVE only:
                # Act is saturated with exp here).  The last 6 units are held
                # back to cover the final head's normalize chain before
                # o_proj(j=1) can start.
                oj0_units = [(nt, c0, cl) for nt in range(KT)
                             for (c0, cl) in _chunks(MC)]

                def _unit_fn(idx):
                    nt, c0, cl = oj0_units[idx]
                    return lambda: emit_oproj_unit(
                        0, nt, c0, cl, gemmps, nc.vector.tensor_copy, idx)

                def _micro():
                    for idx in range(22):
                        yield _unit_fn(idx)
                    while True:
                        yield None

                tail_fill = [[], [], [],
                             [_unit_fn(i) for i in range(22, 32)]]
                emit_attn(1, stps, pvps, tail_fill, order=(1, 3, 0, 2),
                          microfill=_micro())
                emit_oproj(1, gemmps,
                           [nc.vector.tensor_copy, nc.scalar.copy])

    nc.compile()
    return nc


def make_host_inputs(x, w_qkv, w_o):
    """Host-side prep: transpose x, per-core weight slices, rope tables."""
    import ml_dtypes
    bf = ml_dtypes.bfloat16
    x = np.asarray(x, dtype=np.float32)
    w_qkv = np.asarray(w_qkv, dtype=np.float32)
    w_o = np.asarray(w_o, dtype=np.float32)
    xT = np.ascontiguousarray(x.reshape(S, HID).T).astype(bf)

    inv_freq = 1.0 / (ROPE_BASE ** (np.arange(0, HD, 2, dtype=np.float32) / HD))
    t = np.arange(S, dtype=np.float32)
    freqs = np.outer(t, inv_freq)                     # [S, 32]
    cosT = np.cos(freqs).T.astype(np.float32)         # [32, S]
    sinT = np.sin(freqs).T.astype(np.float32)
    C = np.tile(cosT, (4, 1)).astype(bf)              # [128, S]
    Sn = np.tile(np.concatenate([-sinT, sinT], 0), (2, 1)).astype(bf)

    r = np.arange(128)
    maskneg = np.where(r[None, :] < r[:, None], np.float32(NEG),
                       np.float32(0.0)).astype(bf)

    in_maps = []
    for c in range(NCORES):
        qcols = np.arange(4 * c * HD, 4 * (c + 1) * HD)
        vcols = NH * HD + NKV * HD + np.arange(c * HD, (c + 1) * HD)
        kcols = NH * HD + np.arange(c * HD, (c + 1) * HD)
        w_stat = np.ascontiguousarray(
            np.concatenate([w_qkv[:, qcols], w_qkv[:, vcols], w_qkv[:, kcols]],
                           axis=1)).astype(bf)
        w_o_c = np.ascontiguousarray(w_o[256 * c:256 * (c + 1), :]).astype(bf)
        in_maps.append({
            "xT": xT, "w_stat": w_stat, "w_o": w_o_c,
            "C": C, "Sn": Sn, "maskneg": maskneg,
        })
    return in_maps


_NC_CACHE = {}


def get_nc():
    if "nc" not in _NC_CACHE:
        _NC_CACHE["nc"] = build_kernel()
    return _NC_CACHE["nc"]


def _get_exec():
    """Build (once) the jitted sharded executable over the 8 cores."""
    if "exec" in _NC_CACHE:
        return _NC_CACHE["exec"]
    import jax
    from jax.sharding import Mesh, PartitionSpec, NamedSharding
    from jax.experimental.shard_map import shard_map
    from concourse import bass2jax

    nc = get_nc()
    bass2jax.install_neuronx_cc_hook()
    partition_name = (nc.partition_id_tensor.name
                      if nc.partition_id_tensor else None)
    in_names, out_names, out_avals, zero_outs = [], [], [], []
    for alloc in nc.m.functions[0].allocations:
        if not isinstance(alloc, mybir.MemoryLocationSet):
            continue
        name = alloc.memorylocations[0].name
        if alloc.kind == "ExternalInput":
            if name != partition_name:
                in_names.append(name)
        elif alloc.kind == "ExternalOutput":
            shape = tuple(alloc.tensor_shape)
            dtype = mybir.dt.np(alloc.dtype)
            out_names.append(name)
            out_avals.append(jax.core.ShapedArray(shape, dtype))
            zero_outs.append(np.zeros(shape, dtype))
    n_params = len(in_names)
    all_in = list(in_names) + list(out_names)
    if partition_name is not None:
        all_in.append(partition_name)

    def _body(*args):
        operands = list(args)
        if partition_name is not None:
            operands.append(bass2jax.partition_id_tensor())
        return tuple(bass2jax._bass_exec_p.bind(
            *operands, out_avals=tuple(out_avals), in_names=tuple(all_in),
            out_names=tuple(out_names), lowering_input_output_aliases=(),
            sim_require_finite=True, sim_require_nnan=True, nc=nc))

    devices = jax.devices()[:NCORES]
    mesh = Mesh(np.asarray(devices), ("core",))
    REPL = {"xT", "C", "Sn", "maskneg"}
    in_specs = tuple(PartitionSpec() if n in REPL else PartitionSpec("core")
                     for n in in_names)
    in_specs = in_specs + (PartitionSpec("core"),) * len(out_names)
    f = jax.jit(shard_map(_body, mesh=mesh, in_specs=in_specs,
                          out_specs=(PartitionSpec("core"),) * len(out_names),
                          check_rep=False), keep_unused=True)
    sh = NamedSharding(mesh, PartitionSpec("core"))
    shr = NamedSharding(mesh, PartitionSpec())
    _NC_CACHE["exec"] = (f, in_names, out_names, zero_outs, sh, shr, REPL)
    return _NC_CACHE["exec"]


def kernel(x, w_qkv, w_o):
    import jax

    f, in_names, out_names, zero_outs, sh, shr, REPL = _get_exec()
    in_maps = make_host_inputs(x, w_qkv, w_o)
    args = []
    for name in in_names:
        if name in REPL:
            args.append(jax.device_put(in_maps[0][name], shr))
        else:
            args.append(jax.device_put(
                np.concatenate([m[name] for m in in_maps], 0), sh))
    if "zeros" not in _NC_CACHE:
        _NC_CACHE["zeros"] = [
            jax.device_put(
                np.zeros((NCORES * z.shape[0], *z.shape[1:]), z.dtype), sh)
            for z in zero_outs]
    args += _NC_CACHE["zeros"]
    outs = f(*args)
    y_idx = out_names.index("yT")
    if "reduce" not in _NC_CACHE:
        import jax.numpy as jnp
        _NC_CACHE["reduce"] = jax.jit(
            lambda a: jnp.transpose(jnp.sum(
                jnp.reshape(a, (NCORES, HID, S)).astype(jnp.float32),
                axis=0)))
    out = np.asarray(_NC_CACHE["reduce"](outs[y_idx]))
    return np.ascontiguousarray(out.astype(np.float32)).reshape(B, S, HID)


# revision 7
# speedup vs baseline: 1.0342x; 1.0342x over previous
"""GQA causal attention (QKV proj + NeoX RoPE + softmax attention + o_proj)
for Trainium2, tensor-parallel over heads across 8 NeuronCores.  bf16 v2.

Problem shapes (hardcoded): B=1, S=2048, HID=2048, NH=32, NKV=8, HD=64.
Per core c: 4 query heads (4c..4c+3) + 1 kv head (c).

All matmul operands bf16 (1 cyc/row on PE, same as fp32r, but half the DMA
bytes and 4x DVE throughput on SBUF elementwise ops).  PSUM accum fp32.

Single fused schedule (Tile overlaps everything by data deps):
  loads -> qkv(c0) -> rope(c0) -> attn(j0) -> qkv(c1)+rope(c1)
        -> o_proj(j0) -> attn(j1) -> o_proj(j1)
PSUM budget: ph0 pool (6 banks, scoped to c0) released, then
  gemm(2) + st(2x2) + pv(2) = 8 banks.
"""

import numpy as np

import concourse.bass as bass
import concourse.mybir as mybir
import concourse.tile as tile
from concourse import bacc
from concourse.masks import make_identity

B, S, HID = 1, 2048, 2048
NH, NKV, HD = 32, 8, 64
NCORES = 8
HPC = NH // NCORES          # 4 query heads per core
ROPE_BASE = 10000.0
SCALE = 1.0 / np.sqrt(HD)   # 0.125
NEG = -1e9

F32 = mybir.dt.float32
BF16 = mybir.dt.bfloat16

KT = S // 128               # 16 k-tiles
MC = 1024                   # m-chunk / q-chunk
NMC = S // MC               # 2


def _chunks(total, step=512):
    out = []
    o = 0
    while o < total:
        out.append((o, min(step, total - o)))
        o += step
    return out


def build_kernel(passes=1, upto="full"):
    nc = bacc.Bacc("TRN2", target_bir_lowering=False, debug=False,
                   num_devices=NCORES)

    xT = nc.dram_tensor("xT", [HID, S], BF16, kind="ExternalInput").ap()
    w_stat = nc.dram_tensor("w_stat", [HID, 384], BF16,
                            kind="ExternalInput").ap()
    w_o = nc.dram_tensor("w_o", [256, HID], BF16, kind="ExternalInput").ap()
    Cr = nc.dram_tensor("C", [128, S], BF16, kind="ExternalInput").ap()
    Sr = nc.dram_tensor("Sn", [128, S], BF16, kind="ExternalInput").ap()
    maskneg = nc.dram_tensor("maskneg", [128, 128], BF16,
                             kind="ExternalInput").ap()
    yT = nc.dram_tensor("yT", [HID, S], BF16, kind="ExternalOutput").ap()

    with tile.TileContext(nc) as tc:
      for _pass in range(passes):
        with (
            tc.tile_pool(name="pers", bufs=1, side=None) as pers,
            tc.tile_pool(name="swp", bufs=2) as swp,
            tc.tile_pool(name="rtmp", bufs=2) as rtmp,
            tc.tile_pool(name="ptp", bufs=8) as ptp,
            tc.tile_pool(name="sumsp", bufs=2) as sumsp,
            tc.tile_pool(name="recp", bufs=2) as recp,
            tc.tile_pool(name="otp", bufs=2) as otp,
            tc.tile_pool(name="ysbp", bufs=4) as ysbp,
        ):
            # ---------------- persistent tiles ----------------
            wt = [pers.tile([128, 384], BF16, tag=f"wt{k}", name=f"wt{k}")
                  for k in range(KT)]
            xsb = [[pers.tile([128, MC], BF16, tag=f"x{k}_{c}",
                              name=f"x{k}_{c}") for c in range(NMC)]
                   for k in range(KT)]
            # qr[p][c]: roped q heads pair p, chunk c.  kr[c]: rows 64:128
            # roped kT, rows 0:64 duplicate.
            qr = [[pers.tile([128, MC], BF16, tag=f"qr{p}_{c}",
                             name=f"qr{p}_{c}") for c in range(NMC)]
                  for p in range(2)]
            kr = [pers.tile([128, MC], BF16, tag=f"kr{c}", name=f"kr{c}")
                  for c in range(NMC)]
            vaug = [pers.tile([128, 128], BF16, tag=f"va{i}", name=f"va{i}")
                    for i in range(KT)]
            outstat = [[pers.tile([128, MC], BF16, tag=f"os{p}_{j}",
                                  name=f"os{p}_{j}") for j in range(NMC)]
                       for p in range(2)]
            wo_sb = [pers.tile([128, HID], BF16, tag=f"wo{p}", name=f"wo{p}")
                     for p in range(2)]
            Ct = pers.tile([128, S], BF16, tag="Ct")
            St = pers.tile([128, S], BF16, tag="St")
            mneg = pers.tile([128, 128], BF16, tag="mneg")
            ident = pers.tile([128, 128], BF16, tag="ident")
            qkv = [[pers.tile([128, MC], BF16, tag=f"qkv{t}_{c}",
                              name=f"qkv{t}_{c}") for c in range(NMC)]
                   for t in range(3)]

            # ---------------- loads ----------------
            # interleave w and x chunk-0 loads so early k tiles arrive first
            for k in range(KT):
                nc.sync.dma_start(wt[k],
                                  w_stat[128 * k:128 * (k + 1), :])
                eng = nc.sync if k % 2 == 0 else nc.gpsimd
                eng.dma_start(xsb[k][0], xT[128 * k:128 * (k + 1), 0:MC])
            nc.gpsimd.dma_start(mneg, maskneg)
            make_identity(nc, ident)
            for i in range(KT):
                nc.gpsimd.memset(vaug[i][:, 64:128], 1.0)
            for k in range(KT):
                eng = nc.sync if k % 2 == 0 else nc.gpsimd
                eng.dma_start(xsb[k][1], xT[128 * k:128 * (k + 1), MC:S])
            for p in range(2):
                nc.sync.dma_start(wo_sb[p], w_o[128 * p:128 * (p + 1), :])
            nc.gpsimd.dma_start(Ct, Cr)
            nc.gpsimd.dma_start(St, Sr)

            def emit_qkv_n(mc, n, psum_pool, evict_eng):
                """One 128-col slice of qkvT for m-chunk mc."""
                for (c0, cl) in _chunks(MC):
                    ps = psum_pool.tile([128, 512], F32, tag="g",
                                        name=f"qps{mc}_{n}_{c0}")
                    for k in range(KT):
                        nc.tensor.matmul(
                            ps[:, 0:cl],
                            wt[k][:, 128 * n:128 * (n + 1)],
                            xsb[k][mc][:, c0:c0 + cl],
                            start=(k == 0), stop=(k == KT - 1))
                    evict_eng(qkv[n][mc][:, c0:c0 + cl], ps[:, 0:cl])

            def emit_qkv_kmajor(mc, psum_pool, evict_engs):
                """All 6 qkvT psum slices at once: k-major over the first 12
                k-tiles (so the PE can consume each x tile as its DMA lands),
                then per-slice completion so evictions overlap the tail."""
                KSPLIT = 12
                pss = {}
                for n in range(3):
                    for (c0, cl) in _chunks(MC):
                        pss[(n, c0)] = psum_pool.tile(
                            [128, 512], F32, tag="g", name=f"qps{mc}_{n}_{c0}")
                for k in range(KSPLIT):
                    for n in (2, 0, 1):
                        for (c0, cl) in _chunks(MC):
                            nc.tensor.matmul(
                                pss[(n, c0)][:, 0:cl],
                                wt[k][:, 128 * n:128 * (n + 1)],
                                xsb[k][mc][:, c0:c0 + cl],
                                start=(k == 0), stop=False)
                ei = 0
                for n in (2, 0, 1):
                    for (c0, cl) in _chunks(MC):
                        for k in range(KSPLIT, KT):
                            nc.tensor.matmul(
                                pss[(n, c0)][:, 0:cl],
                                wt[k][:, 128 * n:128 * (n + 1)],
                                xsb[k][mc][:, c0:c0 + cl],
                                start=False, stop=(k == KT - 1))
                        evict_engs[ei % len(evict_engs)](
                            qkv[n][mc][:, c0:c0 + cl], pss[(n, c0)][:, 0:cl])
                        ei += 1

            def emit_rope(mc, t, split=False):
                """RoPE for tile t (0,1 = q pairs; 2 = kT rows 64:128).
                split: process column halves separately so the first half of
                qr/kr is ready earlier (attention starts on it sooner)."""
                src = qkv[t][mc]
                r0 = 0 if t < 2 else 64
                m0 = MC * mc
                sw = swp.tile([128, MC], BF16, tag="sw", name=f"sw{t}_{mc}")
                t1 = rtmp.tile([128, MC], BF16, tag="t1", name=f"t1_{t}_{mc}")
                t2 = rtmp.tile([128, MC], BF16, tag="t2", name=f"t2_{t}_{mc}")
                dst = qr[t][mc] if t < 2 else kr[mc]
                for (c0, cl) in _chunks(MC, 512 if split else MC):
                    for g in range(r0 // 32, 4, 2):
                        nc.gpsimd.dma_start(
                            sw[32 * g:32 * g + 32, c0:c0 + cl],
                            src[32 * g + 32:32 * g + 64, c0:c0 + cl])
                        nc.gpsimd.dma_start(
                            sw[32 * g + 32:32 * g + 64, c0:c0 + cl],
                            src[32 * g:32 * g + 32, c0:c0 + cl])
                    nc.vector.tensor_mul(t1[r0:128, c0:c0 + cl],
                                         src[r0:128, c0:c0 + cl],
                                         Ct[r0:128, m0 + c0:m0 + c0 + cl])
                    nc.vector.tensor_mul(t2[r0:128, c0:c0 + cl],
                                         sw[r0:128, c0:c0 + cl],
                                         St[r0:128, m0 + c0:m0 + c0 + cl])
                    nc.vector.tensor_add(dst[r0:128, c0:c0 + cl],
                                         t1[r0:128, c0:c0 + cl],
                                         t2[r0:128, c0:c0 + cl])
                    if t == 2:
                        nc.gpsimd.dma_start(kr[mc][0:64, c0:c0 + cl],
                                            kr[mc][64:128, c0:c0 + cl])

            def emit_transposes(mc, psum_pool):
                for ic in range(8):
                    i = 8 * mc + ic
                    tp = psum_pool.tile([128, 64], BF16, tag="g",
                                        name=f"tr{i}")
                    nc.tensor.transpose(
                        tp, qkv[2][mc][0:64, 128 * ic:128 * (ic + 1)],
                        ident[0:64, 0:64])
                    nc.vector.tensor_copy(vaug[i][:, 0:64], tp)

            def emit_attn(j, stpool, pvpool, fillers=(), order=None,
                          microfill=None):
                """Global 2-deep software pipeline over all (head, i) tiles:
                pv matmuls lag their exp by two st tiles, and a head's
                normalize + boundary fillers fire after the next head's
                first tiles, so neither PE nor Act idles at boundaries."""
                order = list(order) if order is not None else list(range(HPC))
                ilast = 8 * (j + 1) - 1
                tiles = [(hx, h, i) for hx, h in enumerate(order)
                         for i in range(ilast + 1)]
                from collections import deque
                pend = deque()      # (pv, segs, done_fn or None)
                pvs = {}
                nseen = 0

                def flush_one():
                    pv, segs, done_fn = pend.popleft()
                    for (a0, al, pi, ppt, pqoff, stp) in segs:
                        nc.tensor.matmul(
                            pv[:, a0:a0 + al], vaug[pi],
                            ppt[:, a0 - pqoff:a0 - pqoff + al],
                            start=(pi == 0), stop=stp)
                    if done_fn is not None:
                        done_fn()

                def make_norm(hx, h, p, half, pv, last):
                    def norm():
                        rec = sumsp.tile([128, MC], F32, tag="rec",
                                         name=f"rcc{j}_{h}")
                        sums0 = sumsp.tile([64, MC], F32, tag="sums0",
                                           name=f"sm{j}_{h}")
                        rec0 = recp.tile([64, MC], F32, tag="rec0",
                                         name=f"rc{j}_{h}")
                        ot = (None if half == 0 else
                              otp.tile([64, MC], BF16, tag="ot",
                                       name=f"ot{j}_{h}"))
                        cks = _chunks(MC, 512)
                        for (c0, cl) in cks:
                            nc.vector.tensor_copy(rec[64:128, c0:c0 + cl],
                                                  pv[64:128, c0:c0 + cl])
                            nc.gpsimd.dma_start(sums0[:, c0:c0 + cl],
                                                rec[64:128, c0:c0 + cl])
                        for (c0, cl) in cks:
                            nc.vector.reciprocal_approx_fast(
                                rec0[:, c0:c0 + cl], sums0[:, c0:c0 + cl])
                            if half == 0:
                                nc.vector.tensor_mul(
                                    outstat[p][j][0:64, c0:c0 + cl],
                                    pv[0:64, c0:c0 + cl],
                                    rec0[:, c0:c0 + cl])
                            else:
                                nc.vector.tensor_mul(ot[:, c0:c0 + cl],
                                                     pv[0:64, c0:c0 + cl],
                                                     rec0[:, c0:c0 + cl])
                                nc.gpsimd.dma_start(
                                    outstat[p][j][64:128, c0:c0 + cl],
                                    ot[:, c0:c0 + cl])
                        if hx < len(fillers):
                            for fn in fillers[hx]:
                                fn()
                    return norm

                for (hx, h, i) in tiles:
                    p, half = h // 2, h % 2
                    qrow = 64 * half
                    kb = qrow
                    if i == 0:
                        pvs[h] = pvpool.tile([128, MC], F32, tag="pv",
                                             name=f"pv{j}_{h}")
                    pv = pvs[h]
                    kc, ic = i // 8, i % 8
                    qstart = max(MC * j, 128 * i)
                    qlen = MC * (j + 1) - qstart
                    qoff = qstart - MC * j
                    diag = 128 * i >= MC * j
                    st = stpool.tile([128, MC], F32, tag="st",
                                     name=f"st{j}_{h}_{i}")
                    kt_ap = kr[kc][kb:kb + 64, 128 * ic:128 * (ic + 1)]
                    if diag:
                        nc.tensor.matmul(st[:, 0:128], ident, mneg,
                                         start=True, stop=False)
                        nc.tensor.matmul(
                            st[:, 0:128], kt_ap,
                            qr[p][j][qrow:qrow + 64, qoff:qoff + 128],
                            start=False, stop=True)
                        rest = ([(128, min(qlen, 512) - 128)]
                                if min(qlen, 512) > 128 else [])
                        rest += [(c0, cl) for (c0, cl) in _chunks(qlen)
                                 if c0 >= 512]
                    else:
                        rest = _chunks(qlen)
                    for (c0, cl) in rest:
                        nc.tensor.matmul(
                            st[:, c0:c0 + cl], kt_ap,
                            qr[p][j][qrow:qrow + 64,
                                     qoff + c0:qoff + c0 + cl],
                            start=True, stop=True)
                    while len(pend) >= 2:
                        flush_one()
                    pt = ptp.tile([128, MC], BF16, tag="pt",
                                  name=f"pt{j}_{h}_{i}")
                    nc.scalar.activation(
                        pt[:, 0:qlen], st[:, 0:qlen],
                        mybir.ActivationFunctionType.Exp, scale=SCALE)
                    segs, a = [], qoff
                    while a < MC:
                        nxt = min(MC, (a // 512 + 1) * 512)
                        lasti = (8 * j + 3) if a < 512 else ilast
                        segs.append((a, nxt - a, i, pt, qoff, i == lasti))
                        a = nxt
                    done = None
                    if i == ilast:
                        done = make_norm(hx, h, p, half, pv,
                                         hx == len(order) - 1)
                    pend.append((pv, segs, done))
                    nseen += 1
                    if microfill is not None:
                        u = next(microfill, None)
                        if u is not None:
                            u()
                while pend:
                    flush_one()

            ysb_open = {}

            def emit_oproj_unit(j, nt, c0, cl, psum_pool, evict_eng, ei):
                ps = psum_pool.tile([128, 512], F32, tag="g",
                                    name=f"ops{j}_{nt}_{c0}")
                for p in range(2):
                    nc.tensor.matmul(
                        ps[:, 0:cl],
                        wo_sb[p][:, 128 * nt:128 * (nt + 1)],
                        outstat[p][j][:, c0:c0 + cl],
                        start=(p == 0), stop=(p == 1))
                # merge the two 512-col halves into one [128, MC] store
                if (j, nt) not in ysb_open:
                    ysb_open[(j, nt)] = ysbp.tile([128, MC], BF16, tag="y",
                                                  name=f"ysb{j}_{nt}")
                ysb = ysb_open[(j, nt)]
                evict_eng(ysb[:, c0:c0 + cl], ps[:, 0:cl])
                if c0 + cl == MC:
                    del ysb_open[(j, nt)]
                    dmae = nc.sync if ei % 2 == 0 else nc.gpsimd
                    dmae.dma_start(
                        yT[128 * nt:128 * (nt + 1), MC * j:MC * (j + 1)],
                        ysb)

            def emit_oproj(j, psum_pool, evict_engs, nts=None):
                ei = 0
                for nt in (range(KT) if nts is None else nts):
                    for (c0, cl) in _chunks(MC):
                        emit_oproj_unit(j, nt, c0, cl, psum_pool,
                                        evict_engs[ei % len(evict_engs)], ei)
                        ei += 1

            # ---------------- chunk 0 (6-bank scoped psum) ----------------
            with tc.tile_pool(name="ph0ps", bufs=6, space="PSUM") as ph0ps:
                emit_qkv_kmajor(0, ph0ps,
                                [nc.scalar.copy, nc.vector.tensor_copy])
                emit_rope(0, 2, split=True)
                emit_rope(0, 0, split=True)
                emit_transposes(0, ph0ps)
                emit_rope(0, 1)

            # ---------------- rest: 2+4+2 banks ----------------
            with tc.tile_pool(name="gemmps", bufs=2,
                              space="PSUM") as gemmps:
              with (
                tc.tile_pool(name="stps", bufs=2, space="PSUM") as stps,
                tc.tile_pool(name="pvps", bufs=1, space="PSUM") as pvps,
              ):
                # attn j=0 with chunk-1 qkv work interleaved at head ends
                # (PE filler while Act drains the exp stream).
                c1ev = [nc.vector.tensor_copy, nc.scalar.copy]
                c1_fill = [
                    [lambda: emit_qkv_n(1, 2, gemmps, c1ev[0]),
                     lambda: emit_rope(1, 2)],
                    [lambda: emit_qkv_n(1, 0, gemmps, c1ev[1]),
                     lambda: emit_rope(1, 0)],
                    [lambda: emit_transposes(1, gemmps)],
                    [],
                ]
                emit_attn(0, stps, pvps, c1_fill, order=(1, 0, 3, 2))

                # attn j=1 with o_proj(j=0) micro-interleaved into the i-loop
                # (one 2-matmul psum unit per i-tile; evictions on DVE only:
                # Act is saturated with exp here).  The last 6 units are held
                # back to cover the final head's normalize chain before
                # o_proj(j=1) can start.
                oj0_units = [(nt, c0, cl) for nt in range(KT)
                             for (c0, cl) in _chunks(MC)]

                def _unit_fn(idx):
                    nt, c0, cl = oj0_units[idx]
                    return lambda: emit_oproj_unit(
                        0, nt, c0, cl, gemmps, nc.vector.tensor_copy, idx)

                def _micro():
                    # chunk-1 qkv n=1 slice, 2 matmuls per unit (fine-grained
                    # so bursts can't starve the exp stream)
                    for (c0, cl) in _chunks(MC):
                        ps = gemmps.tile([128, 512], F32, tag="g",
                                         name=f"qps1_1_{c0}")
                        for k0 in range(0, KT, 2):
                            def mmu(k0=k0, ps=ps, c0=c0, cl=cl):
                                for k in (k0, k0 + 1):
                                    nc.tensor.matmul(
                                        ps[:, 0:cl],
                                        wt[k][:, 128:256],
                                        xsb[k][1][:, c0:c0 + cl],
                                        start=(k == 0), stop=(k == KT - 1))
                            yield mmu
                        def evu(ps=ps, c0=c0, cl=cl):
                            nc.vector.tensor_copy(qkv[1][1][:, c0:c0 + cl],
                                                  ps[:, 0:cl])
                        yield evu
                    yield lambda: emit_rope(1, 1)
                    for idx in range(22):
                        yield _unit_fn(idx)
                        yield None
                    while True:
                        yield None

                tail_fill = [[], [], [],
                             [_unit_fn(i) for i in range(22, 32)]]
                emit_attn(1, stps, pvps, tail_fill, order=(1, 0, 3, 2),
                          microfill=_micro())
              # st/pv pools closed: o_proj(j1) gets a deep psum pool in the
              # released zone so evictions never gate the tail
              with tc.tile_pool(name="gemmC", bufs=4,
                                space="PSUM") as gemmC:
                emit_oproj(1, gemmC,
                           [nc.vector.tensor_copy, nc.scalar.copy])

    nc.compile()
    return nc


def make_host_inputs(x, w_qkv, w_o):
    """Host-side prep: transpose x, per-core weight slices, rope tables."""
    import ml_dtypes
    bf = ml_dtypes.bfloat16
    x = np.asarray(x, dtype=np.float32)
    w_qkv = np.asarray(w_qkv, dtype=np.float32)
    w_o = np.asarray(w_o, dtype=np.float32)
    xT = np.ascontiguousarray(x.reshape(S, HID).T).astype(bf)

    inv_freq = 1.0 / (ROPE_BASE ** (np.arange(0, HD, 2, dtype=np.float32) / HD))
    t = np.arange(S, dtype=np.float32)
    freqs = np.outer(t, inv_freq)                     # [S, 32]
    cosT = np.cos(freqs).T.astype(np.float32)         # [32, S]
    sinT = np.sin(freqs).T.astype(np.float32)
    C = np.tile(cosT, (4, 1)).astype(bf)              # [128, S]
    Sn = np.tile(np.concatenate([-sinT, sinT], 0), (2, 1)).astype(bf)

    r = np.arange(128)
    maskneg = np.where(r[None, :] < r[:, None], np.float32(NEG),
                       np.float32(0.0)).astype(bf)

    in_maps = []
    for c in range(NCORES):
        qcols = np.arange(4 * c * HD, 4 * (c + 1) * HD)
        vcols = NH * HD + NKV * HD + np.arange(c * HD, (c + 1) * HD)
        kcols = NH * HD + np.arange(c * HD, (c + 1) * HD)
        w_stat = np.ascontiguousarray(
            np.concatenate([w_qkv[:, qcols], w_qkv[:, vcols], w_qkv[:, kcols]],
                           axis=1)).astype(bf)
        w_o_c = np.ascontiguousarray(w_o[256 * c:256 * (c + 1), :]).astype(bf)
        in_maps.append({
            "xT": xT, "w_stat": w_stat, "w_o": w_o_c,
            "C": C, "Sn": Sn, "maskneg": maskneg,
        })
    return in_maps


_NC_CACHE = {}


def get_nc():
    if "nc" not in _NC_CACHE:
        _NC_CACHE["nc"] = build_kernel()
    return _NC_CACHE["nc"]


def _get_exec():
    """Build (once) the jitted sharded executable over the 8 cores."""
    if "exec" in _NC_CACHE:
        return _NC_CACHE["exec"]
    import jax
    from jax.sharding import Mesh, PartitionSpec, NamedSharding
    from jax.experimental.shard_map import shard_map
    from concourse import bass2jax

    nc = get_nc()
    bass2jax.install_neuronx_cc_hook()
    partition_name = (nc.partition_id_tensor.name
                      if nc.partition_id_tensor else None)
    in_names, out_names, out_avals, zero_outs = [], [], [], []
    for alloc in nc.m.functions[0].allocations:
        if not isinstance(alloc, mybir.MemoryLocationSet):
            continue
        name = alloc.memorylocations[0].name
        if alloc.kind == "ExternalInput":
            if name != partition_name:
                in_names.append(name)
        elif alloc.kind == "ExternalOutput":
            shape = tuple(alloc.tensor_shape)
            dtype = mybir.dt.np(alloc.dtype)
            out_names.append(name)
            out_avals.append(jax.core.ShapedArray(shape, dtype))
            zero_outs.append(np.zeros(shape, dtype))
    n_params = len(in_names)
    all_in = list(in_names) + list(out_names)
    if partition_name is not None:
        all_in.append(partition_name)

    def _body(*args):
        operands = list(args)
        if partition_name is not None:
            operands.append(bass2jax.partition_id_tensor())
        return tuple(bass2jax._bass_exec_p.bind(
            *operands, out_avals=tuple(out_avals), in_names=tuple(all_in),
            out_names=tuple(out_names), lowering_input_output_aliases=(),
            sim_require_finite=True, sim_require_nnan=True, nc=nc))

    devices = jax.devices()[:NCORES]
    mesh = Mesh(np.asarray(devices), ("core",))
    REPL = {"xT", "C", "Sn", "maskneg"}
    in_specs = tuple(PartitionSpec() if n in REPL else PartitionSpec("core")
                     for n in in_names)
    in_specs = in_specs + (PartitionSpec("core"),) * len(out_names)
    f = jax.jit(shard_map(_body, mesh=mesh, in_specs=in_specs,
                          out_specs=(PartitionSpec("core"),) * len(out_names),
                          check_rep=False), keep_unused=True)
    sh = NamedSharding(mesh, PartitionSpec("core"))
    shr = NamedSharding(mesh, PartitionSpec())
    _NC_CACHE["exec"] = (f, in_names, out_names, zero_outs, sh, shr, REPL)
    return _NC_CACHE["exec"]


def kernel(x, w_qkv, w_o):
    import jax

    f, in_names, out_names, zero_outs, sh, shr, REPL = _get_exec()
    in_maps = make_host_inputs(x, w_qkv, w_o)
    args = []
    for name in in_names:
        if name in REPL:
            args.append(jax.device_put(in_maps[0][name], shr))
        else:
            args.append(jax.device_put(
                np.concatenate([m[name] for m in in_maps], 0), sh))
    if "zeros" not in _NC_CACHE:
        _NC_CACHE["zeros"] = [
            jax.device_put(
                np.zeros((NCORES * z.shape[0], *z.shape[1:]), z.dtype), sh)
            for z in zero_outs]
    args += _NC_CACHE["zeros"]
    outs = f(*args)
    y_idx = out_names.index("yT")
    if "reduce" not in _NC_CACHE:
        import jax.numpy as jnp
        _NC_CACHE["reduce"] = jax.jit(
            lambda a: jnp.transpose(jnp.sum(
                jnp.reshape(a, (NCORES, HID, S)).astype(jnp.float32),
                axis=0)))
    out = np.asarray(_NC_CACHE["reduce"](outs[y_idx]))
    return np.ascontiguousarray(out.astype(np.float32)).reshape(B, S, HID)
